# revision 9
# baseline (speedup 1.0000x reference)
"""Cross-attention layer (LN -> Q/K proj -> masked softmax -> ctx) on 8 trn2 cores.

Sharding: tensor-parallel over heads. Core c owns heads [2c, 2c+1], i.e. 128
of the 1024 q/k projection output dims. Inputs are replicated except the
weight slices and the V slice of encoder_hidden_states. Outputs (scores slice
per head-pair, ctx slice per 128 dims) are concatenated on host.

Device-side layout strategy per (batch b, local head h):
  - qT [64, 512]  and kT [64, 2048] live transposed (dims on partitions) so
    both S = qT.T @ kT ([q,k] orientation, for the scores output) and
    S^T = kT.T @ qT ([k,q] orientation, for the softmax/ctx path) are single
    matmuls from the same operands.
  - mask -inf is added by DVE during the (mandatory) PSUM->SBUF copy of S.
  - exp uses the per-partition bias operand of ScalarE activation in the S^T
    orientation (mask rows = -30000 -> exp underflows to exactly 0).
  - V carries a baked ones-column so the ctxT matmul's row 64 accumulates the
    softmax denominator for free.
"""

import sys

sys.path.insert(0, "/opt/trn_rl_repo")

import numpy as np

import concourse.bacc as bacc
import concourse.bass as bass
import concourse.tile as tile
from concourse import mybir
from concourse.bass_utils import run_bass_kernel_spmd

B, SQ, SK, H, NH, HD = 4, 512, 2048, 1024, 16, 64
NCORES = 8
HPC = NH // NCORES  # heads per core = 2
DPC = HPC * HD      # projection dims per core = 128
LN_EPS = 1e-12
NEG_BIG = -30000.0  # exp(s + NEG_BIG) == 0.0 exactly in fp32

F32 = mybir.dt.float32
F32R = mybir.dt.float32r


def r(ap):
    return ap.bitcast(F32R)


def build_program():
    nc = bacc.Bacc("TRN2", target_bir_lowering=False, debug=False)

    hs_d = nc.dram_tensor("hs", [B * SQ, H], F32, kind="ExternalInput")
    ehsT_d = nc.dram_tensor("ehsT", [H, B * SK], F32R, kind="ExternalInput")
    ehsv_d = nc.dram_tensor("ehsv", [B, SK, 2 * (HD + 1)], F32R, kind="ExternalInput")
    qwT_d = nc.dram_tensor("qwT", [H, DPC], F32R, kind="ExternalInput")
    kwT_d = nc.dram_tensor("kwT", [H, DPC], F32R, kind="ExternalInput")
    qb_d = nc.dram_tensor("qb", [DPC, 1], F32, kind="ExternalInput")
    kb_d = nc.dram_tensor("kb", [DPC, 1], F32, kind="ExternalInput")
    minf_d = nc.dram_tensor("minf", [B, SK], F32, kind="ExternalInput")
    mbigT_d = nc.dram_tensor("mbigT", [128, B * SK // 128], F32, kind="ExternalInput")
    iden_d = nc.dram_tensor("iden", [128, 128], F32R, kind="ExternalInput")
    sc_d = nc.dram_tensor("scores_part", [B, HPC, SQ, SK], F32, kind="ExternalOutput")
    cx_d = nc.dram_tensor("ctx_part", [B, SQ, DPC], F32, kind="ExternalOutput")

    with tile.TileContext(nc) as tc:
        with (
            tc.tile_pool(name="const", bufs=1) as const,
            tc.tile_pool(name="ln", bufs=2) as ln_pool,
            tc.tile_pool(name="lnst", bufs=4) as lnst,
            tc.tile_pool(name="hsT", bufs=1) as hsT_pool,
            tc.tile_pool(name="ehsT", bufs=2) as ehsT_pool,
            tc.tile_pool(name="qkT", bufs=2) as qkT_pool,
            tc.tile_pool(name="mmat", bufs=2) as mmat_pool,
            tc.tile_pool(name="sout", bufs=2) as sout_pool,
            tc.tile_pool(name="pT", bufs=3) as pT_pool,
            tc.tile_pool(name="v65", bufs=3) as v65_pool,
            tc.tile_pool(name="fin", bufs=2) as fin_pool,
            tc.tile_pool(name="ps_main", bufs=3, space="PSUM") as ps_main,
            tc.tile_pool(name="ps_ctx", bufs=2, space="PSUM") as ps_ctx,
            tc.tile_pool(name="ps_small", bufs=2, space="PSUM") as ps_small,
        ):
            # ---- constants ----
            qwT_sb = const.tile([128, 8, DPC], F32R)
            kwT_sb = const.tile([128, 8, DPC], F32R)
            for hc in range(8):
                nc.sync.dma_start(out=qwT_sb[:, hc, :], in_=qwT_d[hc * 128:(hc + 1) * 128, :])
                nc.sync.dma_start(out=kwT_sb[:, hc, :], in_=kwT_d[hc * 128:(hc + 1) * 128, :])
            qb_sb = const.tile([128, 1], F32)
            kb_sb = const.tile([128, 1], F32)
            nc.sync.dma_start(out=qb_sb, in_=qb_d[:, :])
            nc.sync.dma_start(out=kb_sb, in_=kb_d[:, :])
            iden_sb = const.tile([128, 128], F32R)
            nc.sync.dma_start(out=iden_sb, in_=iden_d[:, :])
            mbigT_sb = const.tile([128, B * SK // 128], F32)
            nc.sync.dma_start(out=mbigT_sb, in_=mbigT_d[:, :])
            eps_sb = const.tile([128, 1], F32)
            nc.vector.memset(eps_sb, LN_EPS)

            for b in range(B):
                # ---- layernorm + on-chip transpose of this batch's tokens ----
                hsT = hsT_pool.tile([128, 8, SQ], F32R)  # [H-slice, hc, tokens]
                for tt in range(4):
                    x_t = ln_pool.tile([128, H], F32, tag="x_t")
                    nc.sync.dma_start(
                        out=x_t, in_=hs_d[b * SQ + tt * 128: b * SQ + (tt + 1) * 128, :]
                    )
                    stats = lnst.tile([128, 2, 6], F32, tag="stats")
                    xg = x_t[:].rearrange("p (s d) -> p s d", s=2)
                    for s in range(2):
                        nc.vector.bn_stats(out=stats[:, s, :], in_=xg[:, s, :])
                    mv = lnst.tile([128, 2], F32, tag="mv")
                    nc.vector.bn_aggr(out=mv, in_=stats)
                    std = lnst.tile([128, 1], F32, tag="std")
                    nc.scalar.activation(
                        out=std, in_=mv[:, 1:2],
                        func=mybir.ActivationFunctionType.Sqrt,
                        bias=eps_sb, scale=1.0,
                    )
                    rstd = lnst.tile([128, 1], F32, tag="rstd")
                    nc.vector.reciprocal(out=rstd, in_=std)
                    xn = ln_pool.tile([128, H], F32R, tag="xn")
                    nc.vector.tensor_scalar(
                        out=xn, in0=x_t,
                        scalar1=mv[:, 0:1], scalar2=rstd,
                        op0=mybir.AluOpType.subtract, op1=mybir.AluOpType.mult,
                    )
                    for hc in range(8):
                        ps_t = ps_small.tile([128, 128], F32, tag="ps_t")
                        nc.tensor.transpose(r(ps_t[:]), xn[:, hc * 128:(hc + 1) * 128], iden_sb[:])
                        nc.scalar.activation(
                            out=hsT[:, hc, tt * 128:(tt + 1) * 128], in_=ps_t,
                            func=mybir.ActivationFunctionType.Copy, bias=0.0, scale=1.0,
                        )

                # ---- q projection: qT_all [128 qdims, 512 tokens] ----
                qT_all = qkT_pool.tile([128, SQ], F32R, tag="qT")
                ps_q = ps_main.tile([128, SQ], F32, tag="mm")
                for hc in range(8):
                    nc.tensor.matmul(
                        ps_q[:], qwT_sb[:, hc, :], hsT[:, hc, :],
                        start=(hc == 0), stop=(hc == 7),
                    )
                nc.vector.tensor_scalar_add(out=qT_all, in0=ps_q, scalar1=qb_sb)

                # ---- k projection: kT_all [128 kdims, 2048 enc tokens] ----
                kT_all = qkT_pool.tile([128, SK], F32R, tag="kT")
                for ch in range(4):
                    ehsT_t = ehsT_pool.tile([128, 8, 512], F32R, tag="ehsT")
                    for hc in range(8):
                        nc.sync.dma_start(
                            out=ehsT_t[:, hc, :],
                            in_=ehsT_d[hc * 128:(hc + 1) * 128,
                                       b * SK + ch * 512: b * SK + (ch + 1) * 512],
                        )
                    ps_k = ps_main.tile([128, 512], F32, tag="mm")
                    for hc in range(8):
                        nc.tensor.matmul(
                            ps_k[:], kwT_sb[:, hc, :], ehsT_t[:, hc, :],
                            start=(hc == 0), stop=(hc == 7),
                        )
                    nc.vector.tensor_scalar_add(
                        out=kT_all[:, ch * 512:(ch + 1) * 512], in0=ps_k, scalar1=kb_sb
                    )

                # ---- materialize -inf mask row broadcast over 128 partitions ----
                minf_mat = mmat_pool.tile([128, SK], F32, tag="minf")
                nc.sync.dma_start(out=minf_mat, in_=minf_d[b: b + 1, :].to_broadcast((128, SK)))

                # ---- scores output: S[q,k] tiles + mask add + DMA out ----
                for h in range(HPC):
                    qs = qT_all[h * HD:(h + 1) * HD, :]
                    ks = kT_all[h * HD:(h + 1) * HD, :]
                    for qt in range(4):
                        s_out = sout_pool.tile([128, SK], F32, tag="s_out")
                        for kc in range(4):
                            ps_s = ps_main.tile([128, 512], F32, tag="mm")
                            nc.tensor.matmul(
                                ps_s[:],
                                qs[:, qt * 128:(qt + 1) * 128],
                                ks[:, kc * 512:(kc + 1) * 512],
                                start=True, stop=True,
                            )
                            nc.vector.tensor_add(
                                out=s_out[:, kc * 512:(kc + 1) * 512],
                                in0=ps_s, in1=minf_mat[:, kc * 512:(kc + 1) * 512],
                            )
                        nc.sync.dma_start(
                            out=sc_d[b, h, qt * 128:(qt + 1) * 128, :], in_=s_out
                        )

                # ---- softmax numerator + ctx accumulation, both heads ----
                ps_c = [ps_ctx.tile([HD + 1, SQ], F32, tag="ctx", name=f"ps_c{h}") for h in range(HPC)]
                for kc in range(16):
                    v65 = v65_pool.tile([128, 2 * (HD + 1)], F32R, tag="v65")
                    nc.sync.dma_start(out=v65, in_=ehsv_d[b, kc * 128:(kc + 1) * 128, :])
                    for h in range(HPC):
                        ps_st = ps_main.tile([128, SQ], F32, tag="mm")
                        nc.tensor.matmul(
                            ps_st[:],
                            kT_all[h * HD:(h + 1) * HD, kc * 128:(kc + 1) * 128],
                            qT_all[h * HD:(h + 1) * HD, :],
                            start=True, stop=True,
                        )
                        pT = pT_pool.tile([128, SQ], F32R, tag="pT")
                        nc.scalar.activation(
                            out=pT, in_=ps_st,
                            func=mybir.ActivationFunctionType.Exp,
                            bias=mbigT_sb[:, b * 16 + kc: b * 16 + kc + 1], scale=1.0,
                        )
                        nc.tensor.matmul(
                            ps_c[h][:],
                            v65[:, h * (HD + 1):(h + 1) * (HD + 1)],
                            pT[:],
                            start=(kc == 0), stop=(kc == 15),
                        )

                # ---- finalize: transpose ctxT, divide by denominator, DMA out ----
                ctxT = [fin_pool.tile([96, SQ], F32R, tag=f"ctxT{h}", name=f"ctxT{h}") for h in range(HPC)]
                for h in range(HPC):
                    nc.scalar.activation(
                        out=ctxT[h][0:HD + 1, :], in_=ps_c[h][:],
                        func=mybir.ActivationFunctionType.Copy, bias=0.0, scale=1.0,
                    )
                for qt in range(4):
                    ctx_f = fin_pool.tile([128, DPC], F32, tag="ctx_f")
                    for h in range(HPC):
                        ps_cth = ps_small.tile([128, 96], F32, tag="ps_t", name=f"ps_cth{h}")
                        nc.tensor.matmul(
                            r(ps_cth[:]),
                            ctxT[h][:, qt * 128:(qt + 1) * 128],
                            iden_sb[0:96, 0:96],
                            is_transpose=True, start=True, stop=True,
                        )
                        rden = fin_pool.tile([128, 1], F32, tag="rden", name=f"rden{h}")
                        nc.vector.reciprocal(out=rden, in_=ps_cth[:, HD:HD + 1])
                        nc.vector.tensor_scalar_mul(
                            out=ctx_f[:, h * HD:(h + 1) * HD],
                            in0=ps_cth[:, 0:HD],
                            scalar1=rden,
                        )
                    nc.sync.dma_start(
                        out=cx_d[b, qt * 128:(qt + 1) * 128, :], in_=ctx_f
                    )
    nc.finalize()
    return nc


_NC_CACHE = []


def _get_program():
    if not _NC_CACHE:
        _NC_CACHE.append(build_program())
    return _NC_CACHE[0]


def prepare_inputs(hidden_states, encoder_hidden_states, encoder_attention_mask,
                   q_w, q_b, k_w, k_b, ln_g, ln_b):
    """Host-side shard prep. Returns in_maps (list of 8 dicts)."""
    f = np.float32
    hs = np.ascontiguousarray(np.asarray(hidden_states, f).reshape(B * SQ, H))
    ehs = np.asarray(encoder_hidden_states, f)
    ehsT = np.ascontiguousarray(ehs.reshape(B * SK, H).T)
    mask = np.asarray(encoder_attention_mask)
    ln_g = np.asarray(ln_g, f); ln_b = np.asarray(ln_b, f)
    q_w = np.asarray(q_w, f); k_w = np.asarray(k_w, f)
    q_b = np.asarray(q_b, f); k_b = np.asarray(k_b, f)

    scale = 1.0 / np.sqrt(HD)
    # fold LN affine and the 1/sqrt(HD) score scale into the q projection
    qw_eff = (q_w * ln_g[None, :]) * scale
    qb_eff = (q_b + q_w @ ln_b) * scale
    kw_eff = k_w * 1.0
    kb_eff = k_b

    minf = np.where(mask == 0, f(-np.inf), f(0.0)).astype(f)          # [B, SK]
    mbig = np.where(mask == 0, f(NEG_BIG), f(0.0)).astype(f)          # [B, SK]
    # [128, 64]: col (b*16 + kc), partition p  ->  mask for key kc*128+p of batch b
    mbigT = np.ascontiguousarray(
        mbig.reshape(B * 16, 128).T
    )
    iden = np.eye(128, dtype=f)

    in_maps = []
    for c in range(NCORES):
        d0 = c * DPC
        qwT = np.ascontiguousarray(qw_eff[d0:d0 + DPC, :].T)   # [H, DPC]
        kwT = np.ascontiguousarray(kw_eff[d0:d0 + DPC, :].T)
        qb_c = np.ascontiguousarray(qb_eff[d0:d0 + DPC]).reshape(DPC, 1)
        kb_c = np.ascontiguousarray(kb_eff[d0:d0 + DPC]).reshape(DPC, 1)
        vpart = ehs[:, :, d0:d0 + DPC]                          # [B, SK, 128]
        ones = np.ones((B, SK, 1), f)
        ehsv = np.ascontiguousarray(
            np.concatenate([vpart[:, :, 0:HD], ones, vpart[:, :, HD:DPC], ones], axis=2)
        )                                                       # [B, SK, 130]
        in_maps.append({
            "hs": hs, "ehsT": ehsT, "ehsv": ehsv,
            "qwT": qwT, "kwT": kwT, "qb": qb_c, "kb": kb_c,
            "minf": minf, "mbigT": mbigT, "iden": iden,
        })
    return in_maps


def assemble_outputs(results):
    scores = np.empty((B, NH, SQ, SK), np.float32)
    ctx = np.empty((B, SQ, H), np.float32)
    for c, res in enumerate(results):
        scores[:, c * HPC:(c + 1) * HPC] = res["scores_part"]
        ctx[:, :, c * DPC:(c + 1) * DPC] = res["ctx_part"]
    return ctx, scores


def kernel(**inputs):
    nc = _get_program()
    in_maps = prepare_inputs(**inputs)
    res = run_bass_kernel_spmd(nc, in_maps, list(range(NCORES)))
    return assemble_outputs(res.results)
